# revision 7
# baseline (speedup 1.0000x reference)
"""EGNN layer on 8 Trainium2 NeuronCores (Bass/Tile).

Strategy: sort edges by destination on the host (pure index preprocessing),
partition the 50000 nodes contiguously across 8 cores (6250 each, padded to
6272 = 49 blocks of 128). Each core processes the edges whose destination
falls in its node range: gathers per-edge endpoint rows (x|pos|pos_init) via
indirect DMA, runs the message MLP feature-major on the PE with PSUM
accumulation, scatters messages into per-node-block PSUM accumulators with a
one-hot matmul (edges sorted by dst => each 128-edge chunk hits one 128-node
block), then runs the node MLP. No collectives needed: dst ranges are
disjoint. Outputs are produced as h^T per core and re-assembled on the host.
"""
import numpy as np

import concourse.bacc as bacc
import concourse.bass as bass
import concourse.tile as tile
from concourse import mybir
from concourse.masks import make_identity

P = 128
N, E, F, EF, OUT = 50000, 800000, 64, 16, 64
NC = 8
NPC = N // NC            # 6250 nodes per core
NBLK = (NPC + P - 1) // P  # 49 blocks of 128 nodes
NPAD = NBLK * P          # 6272
TW = 72                  # table row words: x(64) | pos(3) | pos_init(3) | pad(2)
FP32 = mybir.dt.float32
AX = mybir.AluOpType


def _host_prep(x, edge_index, pos, pos_init, edge_attr):
    src = edge_index[0].astype(np.int64)
    dst = edge_index[1].astype(np.int64)
    order = np.argsort(dst, kind="stable")
    src_s, dst_s, attr_s = src[order], dst[order], edge_attr[order]

    core_of = dst_s // NPC
    # block (of 128 nodes) local to each core
    loc = dst_s - core_of * NPC
    blk = loc // P

    # per (core, block) counts -> uniform chunks-per-block C
    counts = np.zeros((NC, NBLK), np.int64)
    np.add.at(counts, (core_of, blk), 1)
    C = int(np.ceil(counts.max() / P))
    ncols = NBLK * C
    epad = ncols * P

    srcT = np.zeros((NC, P, ncols), np.int32)
    dstT = np.ones((NC, P, ncols), np.int32)   # pad dst idx = 1 (pad src = 0)
    dlocT = np.full((NC, P, ncols), -1.0, np.float32)
    attrT = np.zeros((NC, EF, epad), np.float32)

    for c in range(NC):
        m = core_of == c
        sc, dc, ac, bc = src_s[m], dst_s[m], attr_s[m], blk[m]
        lc = (dc - c * NPC) - bc * P  # dst_local in [0,128)
        # slot index: within each block, edges are contiguous (dst-sorted);
        # place block b's edges at slots [b*C*128, b*C*128 + cnt)
        cnt_b = counts[c]
        start_b = np.arange(NBLK) * C * P
        # position within block: cumulative index among edges of same block
        pos_in_blk = np.zeros(len(bc), np.int64)
        np.subtract(np.arange(len(bc)), np.concatenate(
            [[0], np.cumsum(cnt_b)])[bc], out=pos_in_blk)
        slot = start_b[bc] + pos_in_blk
        srcf = np.zeros(epad, np.int32)
        dstf = np.ones(epad, np.int32)
        dlocf = np.full(epad, -1.0, np.float32)
        attrf = np.zeros((epad, EF), np.float32)
        srcf[slot] = sc
        dstf[slot] = dc
        dlocf[slot] = lc
        attrf[slot] = ac
        srcT[c] = srcf.reshape(ncols, P).T
        dstT[c] = dstf.reshape(ncols, P).T
        dlocT[c] = dlocf.reshape(ncols, P).T
        attrT[c] = attrf.T

    table = np.zeros((N, TW), np.float32)
    table[:, :F] = x
    table[:, F:F + 3] = pos
    table[:, F + 3:F + 6] = pos_init

    xT = np.zeros((NC, F, NPAD), np.float32)
    tabd = np.zeros((NC, NPAD, TW), np.float32)
    for c in range(NC):
        xT[c, :, :NPC] = x[c * NPC:(c + 1) * NPC].T
        tabd[c, :NPC] = table[c * NPC:(c + 1) * NPC]
    return C, ncols, epad, table, srcT, dstT, dlocT, attrT, xT, tabd


def _build_program(C, ncols, epad):
    nc = bacc.Bacc("TRN2", target_bir_lowering=False, debug=False)
    tab = nc.dram_tensor("tab", [N, TW], FP32, kind="ExternalInput")
    srcT_d = nc.dram_tensor("srcT", [P, ncols], mybir.dt.int32, kind="ExternalInput")
    tabd_d = nc.dram_tensor("tabd", [NPAD, TW], FP32, kind="ExternalInput")
    dlocT_d = nc.dram_tensor("dlocT", [P, ncols], FP32, kind="ExternalInput")
    attrT_d = nc.dram_tensor("attrT", [EF, epad], FP32, kind="ExternalInput")
    xT_d = nc.dram_tensor("xT", [F, NPAD], FP32, kind="ExternalInput")
    w_names = {
        "mb1": [F, 1], "mw2": [F, F], "mb2": [F, 1],
        "nw1": [2 * F, F], "nb1": [F, 1], "nw2": [F, OUT], "nb2": [OUT, 1],
        "cw1": [F, F], "cb1": [F, 1], "cw2": [F, 3], "cb2": [3, 1],
    }
    wd = {k: nc.dram_tensor(k, s, FP32, kind="ExternalInput")
          for k, s in w_names.items()}
    wd["mw1"] = nc.dram_tensor("mw1", [145, F], FP32, kind="ExternalInput")
    hT_o = nc.dram_tensor("hT", [OUT, NPAD], FP32, kind="ExternalOutput")
    pos_o = nc.dram_tensor("posu", [NPAD, 3], FP32, kind="ExternalOutput")

    with tile.TileContext(nc) as tc:
        with (
            tc.tile_pool(name="const", bufs=1) as cp,
            tc.tile_pool(name="gather", bufs=12) as gp,
            tc.tile_pool(name="small", bufs=8) as sp,
            tc.tile_pool(name="fm", bufs=4) as fmp,
            tc.tile_pool(name="attr", bufs=2) as ap_,
            tc.tile_pool(name="fin", bufs=2) as fp_,
            tc.tile_pool(name="ps", bufs=6, space="PSUM") as pp,
            tc.tile_pool(name="agg", bufs=2, space="PSUM") as aggp,
        ):
            ident = cp.tile([P, P], FP32)
            make_identity(nc, ident[:])
            iota_i = cp.tile([P, P], mybir.dt.int32)
            nc.gpsimd.iota(iota_i[:], pattern=[[1, P]], base=0, channel_multiplier=0)
            iotaB = cp.tile([P, P], FP32)
            nc.vector.tensor_copy(iotaB[:], iota_i[:])
            ones_c = cp.tile([P, 1], FP32)
            nc.vector.memset(ones_c[:], 1.0)

            # load all indices / weights once
            srcT_t = cp.tile([P, ncols], mybir.dt.int32)
            nc.sync.dma_start(srcT_t[:], srcT_d[:])
            dlocT_t = cp.tile([P, ncols], FP32)
            nc.sync.dma_start(dlocT_t[:], dlocT_d[:])
            wt = {}
            for k, s in w_names.items():
                wt[k] = cp.tile(s, FP32, name=f"w_{k}", tag=f"w_{k}")
                nc.sync.dma_start(wt[k][:], wd[k][:])
            mw1_parts = []
            for nm, (r0, r1) in [("mw1a", (0, F)), ("mw1b", (F, 2 * F)),
                                 ("mw1d", (2 * F, 2 * F + 1)),
                                 ("mw1e", (2 * F + 1, 145))]:
                t = cp.tile([r1 - r0, F], FP32, name=nm, tag=nm)
                nc.sync.dma_start(t[:], wd["mw1"][r0:r1, :])
                mw1_parts.append(t)
            mw1a, mw1b, mw1d, mw1e = mw1_parts
            nw1a = cp.tile([F, F], FP32, name="nw1a", tag="nw1a")
            nc.sync.dma_start(nw1a[:], wd["nw1"][0:F, :])
            nw1b = cp.tile([F, F], FP32, name="nw1b", tag="nw1b")
            nc.sync.dma_start(nw1b[:], wd["nw1"][F:2 * F, :])

            def T(in_ap, m, k):
                """transpose in_[k_part, m_free] -> psum [m, k]"""
                o = pp.tile([m, k], FP32, tag="ps")
                nc.tensor.transpose(o[:], in_ap, ident[:k, :k])
                return o

            for b in range(NBLK):
                attr_blk = ap_.tile([EF, C * P], FP32, tag="attr")
                nc.sync.dma_start(attr_blk[:], attrT_d[:, b * C * P:(b + 1) * C * P])
                nodes_blk = ap_.tile([P, TW], FP32, tag="nodes")
                nc.sync.dma_start(nodes_blk[:], tabd_d[b * P:(b + 1) * P, :])
                agg = aggp.tile([P, 68], FP32, tag="agg")
                for j in range(C):
                    col = b * C + j
                    gs = gp.tile([P, TW], FP32, tag="gs")
                    nc.gpsimd.indirect_dma_start(
                        out=gs[:], out_offset=None, in_=tab[:],
                        in_offset=bass.IndirectOffsetOnAxis(
                            ap=srcT_t[:, col:col + 1], axis=0))
                    # one-hot (edge-major) + its transpose (node-major)
                    oh = gp.tile([P, P], FP32, tag="oh")
                    nc.vector.tensor_scalar(out=oh[:], in0=iotaB[:],
                                            scalar1=dlocT_t[:, col:col + 1],
                                            scalar2=None, op0=AX.is_equal)
                    ohT_ps = T(oh[:], P, P)
                    ohT = gp.tile([P, P], FP32, tag="ohT")
                    nc.vector.tensor_copy(ohT[:], ohT_ps[:])
                    # expand dst-node features per edge: [70,128e] = blk.T @ ohT
                    exp_ps = pp.tile([70, P], FP32, tag="ps")
                    nc.tensor.matmul(exp_ps[:], nodes_blk[:, 0:70], ohT[:],
                                     start=True, stop=True)
                    posdT = sp.tile([6, P], FP32, tag="posdT")
                    nc.vector.tensor_copy(posdT[:], exp_ps[F:F + 6, :])
                    posdEM_ps = T(posdT[:], P, 6)
                    # edge-major pos math
                    dp = sp.tile([P, 3], FP32, tag="dp")
                    nc.vector.tensor_sub(dp[:], gs[:, F:F + 3], posdEM_ps[:, 0:3])
                    scal = sp.tile([P, 16], FP32, tag="scal")
                    nc.vector.tensor_tensor(out=dp[:], in0=dp[:], in1=dp[:], op=AX.mult)
                    nc.vector.tensor_reduce(out=scal[:, 0:1], in_=dp[:],
                                            axis=mybir.AxisListType.X, op=AX.add)
                    dpi = sp.tile([P, 3], FP32, tag="dpi")
                    nc.vector.tensor_sub(dpi[:], gs[:, F + 3:F + 6], posdEM_ps[:, 3:6])
                    dpi2 = sp.tile([P, 3], FP32, tag="dpi2")
                    nc.vector.tensor_tensor(out=dpi2[:], in0=dpi[:], in1=dpi[:], op=AX.mult)
                    nc.vector.tensor_reduce(out=scal[:, 1:2], in_=dpi2[:],
                                            axis=mybir.AxisListType.X, op=AX.add)
                    nc.scalar.sqrt(out=scal[:, 2:3], in_=scal[:, 1:2])
                    nc.vector.reciprocal(out=scal[:, 3:4], in_=scal[:, 2:3])
                    # feature-major transposes
                    xsT_ps = T(gs[:, 0:F], F, P)
                    scT_ps = T(scal[:], 16, P)
                    xsT = fmp.tile([F, P], FP32, tag="xsT")
                    nc.scalar.copy(xsT[:], xsT_ps[:])
                    xdT = fmp.tile([F, P], FP32, tag="xdT")
                    nc.scalar.copy(xdT[:], exp_ps[0:F, :])
                    scT = fmp.tile([16, P], FP32, tag="scT")
                    nc.vector.tensor_copy(scT[:], scT_ps[:])
                    # message MLP layer 1 (accumulate 4 K-slices in PSUM)
                    m1 = pp.tile([F, P], FP32, tag="ps")
                    nc.tensor.matmul(m1[:], mw1a[:], xsT[:], start=True, stop=False)
                    nc.tensor.matmul(m1[:], mw1b[:], xdT[:], start=False, stop=False)
                    nc.tensor.matmul(m1[:], mw1d[:], scT[0:1, :],
                                     start=False, stop=False)
                    nc.tensor.matmul(m1[:], mw1e[:],
                                     attr_blk[:, j * P:(j + 1) * P], start=False, stop=True)
                    r1 = fmp.tile([F, P], FP32, tag="r1")
                    nc.scalar.activation(r1[:], m1[:], mybir.ActivationFunctionType.Relu,
                                         bias=wt["mb1"][:, 0:1])
                    m2 = pp.tile([F, P], FP32, tag="ps")
                    nc.tensor.matmul(m2[:], wt["mw2"][:], r1[:], start=True, stop=True)
                    mij = fmp.tile([F, P], FP32, tag="mij")
                    nc.scalar.activation(mij[:], m2[:], mybir.ActivationFunctionType.Identity,
                                         bias=wt["mb2"][:, 0:1])
                    # coord MLP
                    c1p = pp.tile([F, P], FP32, tag="ps")
                    nc.tensor.matmul(c1p[:], wt["cw1"][:], mij[:], start=True, stop=True)
                    c1 = fmp.tile([F, P], FP32, tag="c1")
                    nc.scalar.activation(c1[:], c1p[:], mybir.ActivationFunctionType.Relu,
                                         bias=wt["cb1"][:, 0:1])
                    cwp = pp.tile([3, P], FP32, tag="ps")
                    nc.tensor.matmul(cwp[:], wt["cw2"][:], c1[:], start=True, stop=True)
                    cww = sp.tile([3, P], FP32, tag="cww")
                    nc.scalar.activation(cww[:], cwp[:], mybir.ActivationFunctionType.Identity,
                                         bias=wt["cb2"][:, 0:1])
                    # scatter payload (edge-major)
                    cwEM = T(cww[:], P, 3)
                    mEM = T(mij[:], P, F)
                    srhs = gp.tile([P, 68], FP32, tag="srhs")
                    nc.vector.tensor_copy(srhs[:, 0:F], mEM[:])
                    nc.vector.tensor_copy(srhs[:, F:F + 1], ones_c[:])
                    nc.vector.tensor_mul(srhs[:, F + 1:F + 4], dpi[:], cwEM[:])
                    nc.vector.tensor_scalar_mul(srhs[:, F + 1:F + 4],
                                                srhs[:, F + 1:F + 4], scal[:, 3:4])
                    nc.tensor.matmul(agg[:], oh[:], srhs[:],
                                     start=(j == 0), stop=(j == C - 1))
                # block finalize
                cntm = fp_.tile([P, 1], FP32, tag="cnt")
                nc.vector.tensor_scalar_max(cntm[:], agg[:, F:F + 1], 1.0)
                rec = fp_.tile([P, 1], FP32, tag="rec")
                nc.vector.reciprocal(rec[:], cntm[:])
                m_i = fp_.tile([P, F], FP32, tag="mi")
                nc.vector.tensor_scalar_mul(m_i[:], agg[:, 0:F], rec[:, 0:1])
                posb = fp_.tile([P, 3], FP32, tag="posb")
                nc.vector.tensor_copy(posb[:], agg[:, F + 1:F + 4])
                nc.sync.dma_start(pos_o[b * P:(b + 1) * P, :], posb[:])
                miT_ps = T(m_i[:], F, P)
                miT = fp_.tile([F, P], FP32, tag="miT")
                nc.scalar.copy(miT[:], miT_ps[:])
                xTb = fp_.tile([F, P], FP32, tag="xTb")
                nc.sync.dma_start(xTb[:], xT_d[:, b * P:(b + 1) * P])
                n1 = pp.tile([F, P], FP32, tag="ps")
                nc.tensor.matmul(n1[:], nw1a[:], xTb[:], start=True, stop=False)
                nc.tensor.matmul(n1[:], nw1b[:], miT[:], start=False, stop=True)
                s1 = fp_.tile([F, P], FP32, tag="s1")
                nc.scalar.activation(s1[:], n1[:], mybir.ActivationFunctionType.Relu,
                                     bias=wt["nb1"][:, 0:1])
                hps = pp.tile([OUT, P], FP32, tag="ps")
                nc.tensor.matmul(hps[:], wt["nw2"][:], s1[:], start=True, stop=True)
                hb = fp_.tile([OUT, P], FP32, tag="hb")
                nc.scalar.activation(hb[:], hps[:], mybir.ActivationFunctionType.Identity,
                                     bias=wt["nb2"][:, 0:1])
                nc.sync.dma_start(hT_o[:, b * P:(b + 1) * P], hb[:])
    nc.compile()
    return nc


_CACHE = {}


def kernel(x, edge_index, pos, pos_init, edge_attr,
           mw1, mb1, mw2, mb2, nw1, nb1, nw2, nb2, cw1, cb1, cw2, cb2):
    x = np.asarray(x, np.float32)
    edge_index = np.asarray(edge_index, np.int32)
    pos = np.asarray(pos, np.float32)
    pos_init = np.asarray(pos_init, np.float32)
    edge_attr = np.asarray(edge_attr, np.float32)
    C, ncols, epad, table, srcT, dstT, dlocT, attrT, xT, tabd = _host_prep(
        x, edge_index, pos, pos_init, edge_attr)

    if C not in _CACHE:
        _CACHE[C] = _build_program(C, ncols, epad)
    nc = _CACHE[C]

    wvals = {
        "mw1": mw1, "mb1": np.reshape(mb1, (F, 1)), "mw2": mw2,
        "mb2": np.reshape(mb2, (F, 1)), "nw1": nw1, "nb1": np.reshape(nb1, (F, 1)),
        "nw2": nw2, "nb2": np.reshape(nb2, (OUT, 1)), "cw1": cw1,
        "cb1": np.reshape(cb1, (F, 1)), "cw2": cw2, "cb2": np.reshape(cb2, (3, 1)),
    }
    wvals = {k: np.ascontiguousarray(v, np.float32) for k, v in wvals.items()}
    in_maps = []
    for c in range(NC):
        m = {"tab": table, "srcT": srcT[c], "dlocT": dlocT[c],
             "attrT": attrT[c], "xT": xT[c], "tabd": tabd[c]}
        m.update(wvals)
        in_maps.append(m)

    from concourse.bass_utils import run_bass_kernel_spmd
    res = run_bass_kernel_spmd(nc, in_maps, core_ids=list(range(NC)))

    h = np.zeros((N, OUT), np.float32)
    pu = np.zeros((N, 3), np.float32)
    for c in range(NC):
        h[c * NPC:(c + 1) * NPC] = res.results[c]["hT"].T[:NPC]
        pu[c * NPC:(c + 1) * NPC] = res.results[c]["posu"][:NPC]
    return h, pu


# revision 8
# speedup vs baseline: 24.9239x; 24.9239x over previous
"""EGNN layer on 8 Trainium2 NeuronCores (Bass/Tile).

Strategy: sort edges by destination on the host (pure index preprocessing),
partition the 50000 nodes contiguously across 8 cores (6250 each, padded to
6272 = 49 blocks of 128). Each core processes the edges whose destination
falls in its node range: gathers per-edge endpoint rows (x|pos|pos_init) via
indirect DMA, runs the message MLP feature-major on the PE with PSUM
accumulation, scatters messages into per-node-block PSUM accumulators with a
one-hot matmul (edges sorted by dst => each 128-edge chunk hits one 128-node
block), then runs the node MLP. No collectives needed: dst ranges are
disjoint. Outputs are produced as h^T per core and re-assembled on the host.
"""
import numpy as np

import concourse.bacc as bacc
import concourse.bass as bass
import concourse.tile as tile
from concourse import mybir
from concourse.masks import make_identity

P = 128
N, E, F, EF, OUT = 50000, 800000, 64, 16, 64
NC = 8
NPC = N // NC            # 6250 nodes per core
NBLK = (NPC + P - 1) // P  # 49 blocks of 128 nodes
NPAD = NBLK * P          # 6272
TW = 72                  # table row words: x(64) | pos(3) | pos_init(3) | pad(2)
FP32 = mybir.dt.float32
AX = mybir.AluOpType


def _host_prep(x, edge_index, pos, pos_init, edge_attr):
    src = edge_index[0].astype(np.int64)
    dst = edge_index[1].astype(np.int64)
    order = np.argsort(dst, kind="stable")
    src_s, dst_s, attr_s = src[order], dst[order], edge_attr[order]

    core_of = dst_s // NPC
    # block (of 128 nodes) local to each core
    loc = dst_s - core_of * NPC
    blk = loc // P

    # per (core, block) counts -> uniform chunks-per-block C
    counts = np.zeros((NC, NBLK), np.int64)
    np.add.at(counts, (core_of, blk), 1)
    C = int(np.ceil(counts.max() / P))
    ncols = NBLK * C
    epad = ncols * P

    srcT = np.zeros((NC, P, ncols), np.int32)
    dstT = np.ones((NC, P, ncols), np.int32)   # pad dst idx = 1 (pad src = 0)
    dlocT = np.full((NC, P, ncols), -1.0, np.float32)
    attrT = np.zeros((NC, EF, epad), np.float32)

    for c in range(NC):
        m = core_of == c
        sc, dc, ac, bc = src_s[m], dst_s[m], attr_s[m], blk[m]
        lc = (dc - c * NPC) - bc * P  # dst_local in [0,128)
        # slot index: within each block, edges are contiguous (dst-sorted);
        # place block b's edges at slots [b*C*128, b*C*128 + cnt)
        cnt_b = counts[c]
        start_b = np.arange(NBLK) * C * P
        # position within block: cumulative index among edges of same block
        pos_in_blk = np.zeros(len(bc), np.int64)
        np.subtract(np.arange(len(bc)), np.concatenate(
            [[0], np.cumsum(cnt_b)])[bc], out=pos_in_blk)
        slot = start_b[bc] + pos_in_blk
        srcf = np.zeros(epad, np.int32)
        dstf = np.ones(epad, np.int32)
        dlocf = np.full(epad, -1.0, np.float32)
        attrf = np.zeros((epad, EF), np.float32)
        srcf[slot] = sc
        dstf[slot] = dc
        dlocf[slot] = lc
        attrf[slot] = ac
        srcT[c] = srcf.reshape(ncols, P).T
        dstT[c] = dstf.reshape(ncols, P).T
        dlocT[c] = dlocf.reshape(ncols, P).T
        attrT[c] = attrf.T

    table = np.zeros((N, TW), np.float32)
    table[:, :F] = x
    table[:, F:F + 3] = pos
    table[:, F + 3:F + 6] = pos_init

    xT = np.zeros((NC, F, NPAD), np.float32)
    tabd = np.zeros((NC, NPAD, TW), np.float32)
    for c in range(NC):
        xT[c, :, :NPC] = x[c * NPC:(c + 1) * NPC].T
        tabd[c, :NPC] = table[c * NPC:(c + 1) * NPC]
    return C, ncols, epad, table, srcT, dstT, dlocT, attrT, xT, tabd


def _build_program(C, ncols, epad):
    nc = bacc.Bacc("TRN2", target_bir_lowering=False, debug=False)
    tab = nc.dram_tensor("tab", [N, TW], FP32, kind="ExternalInput")
    srcT_d = nc.dram_tensor("srcT", [P, ncols], mybir.dt.int32, kind="ExternalInput")
    dstT_d = nc.dram_tensor("dstT", [P, ncols], mybir.dt.int32, kind="ExternalInput")
    dlocT_d = nc.dram_tensor("dlocT", [P, ncols], FP32, kind="ExternalInput")
    attrT_d = nc.dram_tensor("attrT", [EF, epad], FP32, kind="ExternalInput")
    xT_d = nc.dram_tensor("xT", [F, NPAD], FP32, kind="ExternalInput")
    w_names = {
        "mb1": [F, 1], "mw2": [F, F], "mb2": [F, 1],
        "nw1": [2 * F, F], "nb1": [F, 1], "nw2": [F, OUT], "nb2": [OUT, 1],
        "cw1": [F, F], "cb1": [F, 1], "cw2": [F, 3], "cb2": [3, 1],
    }
    wd = {k: nc.dram_tensor(k, s, FP32, kind="ExternalInput")
          for k, s in w_names.items()}
    wd["mw1"] = nc.dram_tensor("mw1", [145, F], FP32, kind="ExternalInput")
    hT_o = nc.dram_tensor("hT", [OUT, NPAD], FP32, kind="ExternalOutput")
    pos_o = nc.dram_tensor("posu", [NPAD, 3], FP32, kind="ExternalOutput")

    with tile.TileContext(nc) as tc:
        with (
            tc.tile_pool(name="const", bufs=1) as cp,
            tc.tile_pool(name="gather", bufs=12) as gp,
            tc.tile_pool(name="small", bufs=8) as sp,
            tc.tile_pool(name="fm", bufs=4) as fmp,
            tc.tile_pool(name="attr", bufs=2) as ap_,
            tc.tile_pool(name="fin", bufs=2) as fp_,
            tc.tile_pool(name="ps", bufs=6, space="PSUM") as pp,
            tc.tile_pool(name="agg", bufs=2, space="PSUM") as aggp,
        ):
            ident = cp.tile([P, P], FP32)
            make_identity(nc, ident[:])
            iota_i = cp.tile([P, P], mybir.dt.int32)
            nc.gpsimd.iota(iota_i[:], pattern=[[1, P]], base=0, channel_multiplier=0)
            iotaB = cp.tile([P, P], FP32)
            nc.vector.tensor_copy(iotaB[:], iota_i[:])
            ones_c = cp.tile([P, 1], FP32)
            nc.vector.memset(ones_c[:], 1.0)

            # load all indices / weights once
            srcT_t = cp.tile([P, ncols], mybir.dt.int32)
            nc.sync.dma_start(srcT_t[:], srcT_d[:])
            dstT_t = cp.tile([P, ncols], mybir.dt.int32)
            nc.sync.dma_start(dstT_t[:], dstT_d[:])
            dlocT_t = cp.tile([P, ncols], FP32)
            nc.sync.dma_start(dlocT_t[:], dlocT_d[:])
            wt = {}
            for k, s in w_names.items():
                wt[k] = cp.tile(s, FP32, name=f"w_{k}", tag=f"w_{k}")
                nc.sync.dma_start(wt[k][:], wd[k][:])
            mw1_parts = []
            for nm, (r0, r1) in [("mw1a", (0, F)), ("mw1b", (F, 2 * F)),
                                 ("mw1d", (2 * F, 2 * F + 1)),
                                 ("mw1e", (2 * F + 1, 145))]:
                t = cp.tile([r1 - r0, F], FP32, name=nm, tag=nm)
                nc.sync.dma_start(t[:], wd["mw1"][r0:r1, :])
                mw1_parts.append(t)
            mw1a, mw1b, mw1d, mw1e = mw1_parts
            nw1a = cp.tile([F, F], FP32, name="nw1a", tag="nw1a")
            nc.sync.dma_start(nw1a[:], wd["nw1"][0:F, :])
            nw1b = cp.tile([F, F], FP32, name="nw1b", tag="nw1b")
            nc.sync.dma_start(nw1b[:], wd["nw1"][F:2 * F, :])

            def T(in_ap, m, k):
                """transpose in_[k_part, m_free] -> psum [m, k]"""
                o = pp.tile([m, k], FP32, tag="ps")
                nc.tensor.transpose(o[:], in_ap, ident[:k, :k])
                return o

            for b in range(NBLK):
                attr_blk = ap_.tile([EF, C * P], FP32, tag="attr")
                nc.sync.dma_start(attr_blk[:], attrT_d[:, b * C * P:(b + 1) * C * P])
                agg = aggp.tile([P, 68], FP32, tag="agg")
                for j in range(C):
                    col = b * C + j
                    gs = gp.tile([P, TW], FP32, tag="gs")
                    nc.gpsimd.indirect_dma_start(
                        out=gs[:], out_offset=None, in_=tab[:],
                        in_offset=bass.IndirectOffsetOnAxis(
                            ap=srcT_t[:, col:col + 1], axis=0))
                    gd = gp.tile([P, TW], FP32, tag="gd")
                    nc.gpsimd.indirect_dma_start(
                        out=gd[:], out_offset=None, in_=tab[:],
                        in_offset=bass.IndirectOffsetOnAxis(
                            ap=dstT_t[:, col:col + 1], axis=0))
                    # edge-major pos math
                    dp = sp.tile([P, 3], FP32, tag="dp")
                    nc.vector.tensor_sub(dp[:], gs[:, F:F + 3], gd[:, F:F + 3])
                    scal = sp.tile([P, 16], FP32, tag="scal")
                    nc.vector.tensor_tensor(out=dp[:], in0=dp[:], in1=dp[:], op=AX.mult)
                    nc.vector.tensor_reduce(out=scal[:, 0:1], in_=dp[:],
                                            axis=mybir.AxisListType.X, op=AX.add)
                    dpi = sp.tile([P, 3], FP32, tag="dpi")
                    nc.vector.tensor_sub(dpi[:], gs[:, F + 3:F + 6], gd[:, F + 3:F + 6])
                    dpi2 = sp.tile([P, 3], FP32, tag="dpi2")
                    nc.vector.tensor_tensor(out=dpi2[:], in0=dpi[:], in1=dpi[:], op=AX.mult)
                    nc.vector.tensor_reduce(out=scal[:, 1:2], in_=dpi2[:],
                                            axis=mybir.AxisListType.X, op=AX.add)
                    nc.scalar.sqrt(out=scal[:, 2:3], in_=scal[:, 1:2])
                    nc.vector.reciprocal(out=scal[:, 3:4], in_=scal[:, 2:3])
                    # feature-major transposes
                    xsT_ps = T(gs[:, 0:F], F, P)
                    xdT_ps = T(gd[:, 0:F], F, P)
                    scT_ps = T(scal[:], 16, P)
                    xsT = fmp.tile([F, P], FP32, tag="xsT")
                    nc.scalar.copy(xsT[:], xsT_ps[:])
                    xdT = fmp.tile([F, P], FP32, tag="xdT")
                    nc.scalar.copy(xdT[:], xdT_ps[:])
                    scT = fmp.tile([16, P], FP32, tag="scT")
                    nc.vector.tensor_copy(scT[:], scT_ps[:])
                    # message MLP layer 1 (accumulate 4 K-slices in PSUM)
                    m1 = pp.tile([F, P], FP32, tag="ps")
                    nc.tensor.matmul(m1[:], mw1a[:], xsT[:], start=True, stop=False)
                    nc.tensor.matmul(m1[:], mw1b[:], xdT[:], start=False, stop=False)
                    nc.tensor.matmul(m1[:], mw1d[:], scT[0:1, :],
                                     start=False, stop=False)
                    nc.tensor.matmul(m1[:], mw1e[:],
                                     attr_blk[:, j * P:(j + 1) * P], start=False, stop=True)
                    r1 = fmp.tile([F, P], FP32, tag="r1")
                    nc.scalar.activation(r1[:], m1[:], mybir.ActivationFunctionType.Relu,
                                         bias=wt["mb1"][:, 0:1])
                    m2 = pp.tile([F, P], FP32, tag="ps")
                    nc.tensor.matmul(m2[:], wt["mw2"][:], r1[:], start=True, stop=True)
                    mij = fmp.tile([F, P], FP32, tag="mij")
                    nc.scalar.activation(mij[:], m2[:], mybir.ActivationFunctionType.Identity,
                                         bias=wt["mb2"][:, 0:1])
                    # coord MLP
                    c1p = pp.tile([F, P], FP32, tag="ps")
                    nc.tensor.matmul(c1p[:], wt["cw1"][:], mij[:], start=True, stop=True)
                    c1 = fmp.tile([F, P], FP32, tag="c1")
                    nc.scalar.activation(c1[:], c1p[:], mybir.ActivationFunctionType.Relu,
                                         bias=wt["cb1"][:, 0:1])
                    cwp = pp.tile([3, P], FP32, tag="ps")
                    nc.tensor.matmul(cwp[:], wt["cw2"][:], c1[:], start=True, stop=True)
                    cww = sp.tile([3, P], FP32, tag="cww")
                    nc.scalar.activation(cww[:], cwp[:], mybir.ActivationFunctionType.Identity,
                                         bias=wt["cb2"][:, 0:1])
                    # scatter payload (edge-major)
                    cwEM = T(cww[:], P, 3)
                    mEM = T(mij[:], P, F)
                    srhs = gp.tile([P, 68], FP32, tag="srhs")
                    nc.vector.tensor_copy(srhs[:, 0:F], mEM[:])
                    nc.vector.tensor_copy(srhs[:, F:F + 1], ones_c[:])
                    nc.vector.tensor_mul(srhs[:, F + 1:F + 4], dpi[:], cwEM[:])
                    nc.vector.tensor_scalar_mul(srhs[:, F + 1:F + 4],
                                                srhs[:, F + 1:F + 4], scal[:, 3:4])
                    oh = gp.tile([P, P], FP32, tag="oh")
                    nc.vector.tensor_scalar(out=oh[:], in0=iotaB[:],
                                            scalar1=dlocT_t[:, col:col + 1],
                                            scalar2=None, op0=AX.is_equal)
                    nc.tensor.matmul(agg[:], oh[:], srhs[:],
                                     start=(j == 0), stop=(j == C - 1))
                # block finalize
                cntm = fp_.tile([P, 1], FP32, tag="cnt")
                nc.vector.tensor_scalar_max(cntm[:], agg[:, F:F + 1], 1.0)
                rec = fp_.tile([P, 1], FP32, tag="rec")
                nc.vector.reciprocal(rec[:], cntm[:])
                m_i = fp_.tile([P, F], FP32, tag="mi")
                nc.vector.tensor_scalar_mul(m_i[:], agg[:, 0:F], rec[:, 0:1])
                posb = fp_.tile([P, 3], FP32, tag="posb")
                nc.vector.tensor_copy(posb[:], agg[:, F + 1:F + 4])
                nc.sync.dma_start(pos_o[b * P:(b + 1) * P, :], posb[:])
                miT_ps = T(m_i[:], F, P)
                miT = fp_.tile([F, P], FP32, tag="miT")
                nc.scalar.copy(miT[:], miT_ps[:])
                xTb = fp_.tile([F, P], FP32, tag="xTb")
                nc.sync.dma_start(xTb[:], xT_d[:, b * P:(b + 1) * P])
                n1 = pp.tile([F, P], FP32, tag="ps")
                nc.tensor.matmul(n1[:], nw1a[:], xTb[:], start=True, stop=False)
                nc.tensor.matmul(n1[:], nw1b[:], miT[:], start=False, stop=True)
                s1 = fp_.tile([F, P], FP32, tag="s1")
                nc.scalar.activation(s1[:], n1[:], mybir.ActivationFunctionType.Relu,
                                     bias=wt["nb1"][:, 0:1])
                hps = pp.tile([OUT, P], FP32, tag="ps")
                nc.tensor.matmul(hps[:], wt["nw2"][:], s1[:], start=True, stop=True)
                hb = fp_.tile([OUT, P], FP32, tag="hb")
                nc.scalar.activation(hb[:], hps[:], mybir.ActivationFunctionType.Identity,
                                     bias=wt["nb2"][:, 0:1])
                nc.sync.dma_start(hT_o[:, b * P:(b + 1) * P], hb[:])
    nc.compile()
    return nc


_CACHE = {}


def kernel(x, edge_index, pos, pos_init, edge_attr,
           mw1, mb1, mw2, mb2, nw1, nb1, nw2, nb2, cw1, cb1, cw2, cb2):
    x = np.asarray(x, np.float32)
    edge_index = np.asarray(edge_index, np.int32)
    pos = np.asarray(pos, np.float32)
    pos_init = np.asarray(pos_init, np.float32)
    edge_attr = np.asarray(edge_attr, np.float32)
    C, ncols, epad, table, srcT, dstT, dlocT, attrT, xT, tabd = _host_prep(
        x, edge_index, pos, pos_init, edge_attr)

    if C not in _CACHE:
        _CACHE[C] = _build_program(C, ncols, epad)
    nc = _CACHE[C]

    wvals = {
        "mw1": mw1, "mb1": np.reshape(mb1, (F, 1)), "mw2": mw2,
        "mb2": np.reshape(mb2, (F, 1)), "nw1": nw1, "nb1": np.reshape(nb1, (F, 1)),
        "nw2": nw2, "nb2": np.reshape(nb2, (OUT, 1)), "cw1": cw1,
        "cb1": np.reshape(cb1, (F, 1)), "cw2": cw2, "cb2": np.reshape(cb2, (3, 1)),
    }
    wvals = {k: np.ascontiguousarray(v, np.float32) for k, v in wvals.items()}
    in_maps = []
    for c in range(NC):
        m = {"tab": table, "srcT": srcT[c], "dstT": dstT[c], "dlocT": dlocT[c],
             "attrT": attrT[c], "xT": xT[c]}
        m.update(wvals)
        in_maps.append(m)

    from concourse.bass_utils import run_bass_kernel_spmd
    res = run_bass_kernel_spmd(nc, in_maps, core_ids=list(range(NC)))

    h = np.zeros((N, OUT), np.float32)
    pu = np.zeros((N, 3), np.float32)
    for c in range(NC):
        h[c * NPC:(c + 1) * NPC] = res.results[c]["hT"].T[:NPC]
        pu[c * NPC:(c + 1) * NPC] = res.results[c]["posu"][:NPC]
    return h, pu
